# revision 16
# baseline (speedup 1.0000x reference)
"""KNN segmentator kernel for 8 TRN2 NeuronCores.

Strategy (sharding_hint): shard the train bank along N (2048 -> 8 x 256).
Launch 1 (SPMD, 8 cores): each core computes sim = einsum('bpd,pdn->bpn')
for its N-window via PE matmuls (4 patches col-tiled per PSUM tile), then
local top-24 per (b,p) row with DVE max8/max_index/match_replace.
Host: merge 8x24 candidates -> global top-20 (jax top_k tie semantics).
Launch 2 (SPMD, 8 cores): gather neighbor label columns with indirect DMA
from a p-sharded, pre-transposed label bank; host assembles the output grid.
"""

import os

import numpy as np

import concourse.bacc as bacc
import concourse.bass as bass
import concourse.mybir as mybir
import concourse.tile as tile
import concourse.bass_utils as bass_utils

B, P, D, N, K, PS = 16, 196, 768, 2048, 20, 16
NCORES = 8
NL = N // NCORES          # 256 bank columns per core
NQ = P // 4               # 49 quads of 4 patches
NCH = D // 128            # 6 contraction chunks
KCAND = 24                # local top-24 (3 rounds of max8)
PPAD = 200                # p padded for launch-2 sharding
PL = PPAD // NCORES       # 25 patches per core in launch 2
GROWS = B * K             # 320 gathered rows per patch
NGR = PL * GROWS          # 8000 gathered rows per core
GTILES = (NGR + 127) // 128    # 63 indirect-DMA tiles of <=128 rows

_cache = {}
LAST_HW_NS = None


FP32_MATMUL = bool(os.environ.get("KERNEL_FP32"))


def _build_launch1():
    nc = bacc.Bacc("TRN2", target_bir_lowering=False, debug=False,
                   num_devices=NCORES)
    if FP32_MATMUL:
        xt_d = nc.dram_tensor("xt", [128, P, NCH, B], mybir.dt.float32,
                              kind="ExternalInput")
        w_d = nc.dram_tensor("w", [NQ, 128, 4, NCH, NL], mybir.dt.float32,
                             kind="ExternalInput")
    else:
        # fp16 hi/lo split: x = xh + xl, w = wh + wl (f16 pairs, fp32-class
        # accuracy from 3 accumulated products: xh*wh + xh*wl + xl*wh)
        xt_d = nc.dram_tensor("xt", [128, P, NCH, 2, B], mybir.dt.float16,
                              kind="ExternalInput")
        w_d = nc.dram_tensor("w", [NQ, 128, 4, NCH, 2, NL], mybir.dt.float16,
                             kind="ExternalInput")
    vals_d = nc.dram_tensor("vals", [NQ, 128, KCAND], mybir.dt.float32,
                            kind="ExternalOutput")
    idxs_d = nc.dram_tensor("idxs", [NQ, 128, KCAND], mybir.dt.uint32,
                            kind="ExternalOutput")

    with tile.TileContext(nc) as tc:
        with (
            tc.tile_pool(name="xp", bufs=4) as xp,
            tc.tile_pool(name="wp", bufs=4) as wp,
            tc.tile_pool(name="sp", bufs=3) as sp,
            tc.tile_pool(name="op", bufs=4) as op,
            tc.tile_pool(name="pp", bufs=4, space="PSUM") as pp,
        ):
            for q in range(NQ):
                if FP32_MATMUL:
                    xq = xp.tile([128, 4, NCH, B], mybir.dt.float32)
                    nc.scalar.dma_start(xq[:], xt_d[:, 4 * q:4 * q + 4, :, :])
                    wq = wp.tile([128, 4, NCH, NL], mybir.dt.float32)
                    nc.sync.dma_start(wq[:], w_d[q])
                else:
                    xq = xp.tile([128, 4, NCH, 2, B], mybir.dt.float16)
                    nc.scalar.dma_start(xq[:], xt_d[:, 4 * q:4 * q + 4])
                    wq = wp.tile([128, 4, NCH, 2, NL], mybir.dt.float16)
                    weng = nc.sync if q % 2 == 0 else nc.scalar
                    weng.dma_start(wq[:], w_d[q])

                pt = pp.tile([128, NL], mybir.dt.float32)
                for j in range(4):
                    if FP32_MATMUL:
                        for c in range(NCH):
                            nc.tensor.matmul(
                                pt[32 * j:32 * j + B, :],
                                lhsT=xq[:, j, c, :],
                                rhs=wq[:, j, c, :],
                                start=(c == 0),
                                stop=(c == NCH - 1),
                                tile_position=(0, 32 * j),
                            )
                    else:
                        terms = [(0, 0), (0, 1), (1, 0)]  # (x half, w half)
                        nmm = NCH * len(terms)
                        mi = 0
                        for c in range(NCH):
                            for xh, wh in terms:
                                nc.tensor.matmul(
                                    pt[32 * j:32 * j + B, :],
                                    lhsT=xq[:, j, c, xh, :],
                                    rhs=wq[:, j, c, wh, :],
                                    start=(mi == 0),
                                    stop=(mi == nmm - 1),
                                    tile_position=(0, 32 * j),
                                )
                                mi += 1

                s0 = sp.tile([128, NL], mybir.dt.float32)
                nc.scalar.activation(s0[:], pt[:],
                                     mybir.ActivationFunctionType.Copy)

                v24 = op.tile([128, KCAND], mybir.dt.float32)
                i24 = op.tile([128, KCAND], mybir.dt.uint32)
                s1 = sp.tile([128, NL], mybir.dt.float32)
                s2 = sp.tile([128, NL], mybir.dt.float32)
                nc.vector.max(out=v24[:, 0:8], in_=s0[:])
                nc.vector.max_index(out=i24[:, 0:8], in_max=v24[:, 0:8],
                                    in_values=s0[:])
                nc.vector.match_replace(out=s1[:], in_to_replace=v24[:, 0:8],
                                        in_values=s0[:], imm_value=-1e30)
                nc.vector.max(out=v24[:, 8:16], in_=s1[:])
                nc.vector.max_index(out=i24[:, 8:16], in_max=v24[:, 8:16],
                                    in_values=s1[:])
                nc.vector.match_replace(out=s2[:], in_to_replace=v24[:, 8:16],
                                        in_values=s1[:], imm_value=-1e30)
                nc.vector.max(out=v24[:, 16:24], in_=s2[:])
                nc.vector.max_index(out=i24[:, 16:24], in_max=v24[:, 16:24],
                                    in_values=s2[:])

                nc.scalar.dma_start(vals_d[q], v24[:])
                nc.scalar.dma_start(idxs_d[q], i24[:])

    nc.compile()
    return nc


def _build_launch2():
    nc = bacc.Bacc("TRN2", target_bir_lowering=False, debug=False,
                   num_devices=NCORES)
    # flat label table: row p*N + n  ->  256 labels (pre-transposed on host)
    lab_d = nc.dram_tensor("lab", [PL * N, PS * PS], mybir.dt.int32,
                           kind="ExternalInput")
    off_d = nc.dram_tensor("off", [128, GTILES], mybir.dt.int32,
                           kind="ExternalInput")
    out_d = nc.dram_tensor("gout", [GTILES, 128, PS * PS], mybir.dt.int32,
                           kind="ExternalOutput")

    with tile.TileContext(nc) as tc:
        with tc.tile_pool(name="gp", bufs=8) as gp:
            offt = gp.tile([128, GTILES], mybir.dt.int32, tag="off")
            nc.scalar.dma_start(offt[:], off_d[:])
            for t in range(GTILES):
                gt = gp.tile([128, PS * PS], mybir.dt.int32, tag="g")
                nc.gpsimd.indirect_dma_start(
                    out=gt[:],
                    out_offset=None,
                    in_=lab_d[:],
                    in_offset=bass.IndirectOffsetOnAxis(
                        ap=offt[:, t:t + 1], axis=0),
                )
                nc.sync.dma_start(out_d[t], gt[:])

    nc.compile()
    return nc


def _hilo(a):
    """fp32 array -> fp16 (hi, lo) stacked on a new axis before the last."""
    hi = a.astype(np.float16)
    lo = (a - hi.astype(np.float32)).astype(np.float16)
    return np.stack([hi, lo], axis=-2)


def _prep_inputs(test_feature, train_features):
    # xt[d, p, c, b] = test_feature[b, p, 128c + d]
    xt = np.ascontiguousarray(
        test_feature.reshape(B, P, NCH, 128).transpose(3, 1, 2, 0))
    # per-core w[c8][q, d, j, c, n] = train_features[4q+j, 128c+d, 256*c8+n]
    tf7 = train_features.reshape(NQ, 4, NCH, 128, NCORES, NL)
    w8 = np.ascontiguousarray(tf7.transpose(4, 0, 3, 1, 2, 5))
    if not FP32_MATMUL:
        xt = _hilo(xt)          # [128, P, NCH, 2, B]
        w8 = _hilo(w8)          # [8, NQ, 128, 4, NCH, 2, NL]
    return xt, w8


def _merge_topk(vals, idxs):
    """vals/idxs: [8][NQ,128,24] device outputs -> dist [B,P,K], gidx [B,P,K]."""
    v = vals.reshape(NCORES, NQ, 4, 32, KCAND)[:, :, :, :B, :]
    i = idxs.reshape(NCORES, NQ, 4, 32, KCAND)[:, :, :, :B, :].astype(np.int64)
    i = i + (np.arange(NCORES, dtype=np.int64) * NL)[:, None, None, None, None]
    # -> [b, p=(q,j), cand = c*24]
    v2 = np.ascontiguousarray(v.transpose(3, 1, 2, 0, 4)).reshape(B, P, -1)
    i2 = np.ascontiguousarray(i.transpose(3, 1, 2, 0, 4)).reshape(B, P, -1)
    # sort by (value desc, index asc) == jax.lax.top_k tie order
    order = np.lexsort((i2, -v2), axis=-1)
    vs = np.take_along_axis(v2, order, axis=-1)
    is_ = np.take_along_axis(i2, order, axis=-1)
    # drop exact (v, i) duplicates (max_index tie artifact), keep sort order
    dup = np.zeros_like(vs, dtype=bool)
    dup[:, :, 1:] = (vs[:, :, 1:] == vs[:, :, :-1]) & \
                    (is_[:, :, 1:] == is_[:, :, :-1])
    sel = np.argsort(dup, axis=-1, kind="stable")[:, :, :K]
    dist = np.take_along_axis(vs, sel, axis=-1).astype(np.float32)
    gidx = np.take_along_axis(is_, sel, axis=-1)
    return dist, gidx


def _grid_from_retrieved(g):
    # g: [B, P, K, PS*PS] labels -> [B, K, 14*PS, 14*PS]
    nr = int(np.sqrt(P))
    r = g.reshape(B, nr, nr, K, PS, PS)
    return np.ascontiguousarray(
        r.transpose(0, 3, 1, 4, 2, 5)).reshape(B, K, nr * PS, nr * PS)


USE_DEVICE_GATHER = not bool(os.environ.get("KERNEL_HOST_GATHER"))


def kernel(test_feature, train_features, train_labels):
    global LAST_HW_NS
    test_feature = np.asarray(test_feature, dtype=np.float32)
    train_features = np.asarray(train_features, dtype=np.float32)
    train_labels = np.asarray(train_labels, dtype=np.int32)

    trace = bool(os.environ.get("KERNEL_TRACE"))
    kw = dict(trace=True, trace_cores=list(range(NCORES))) if trace else {}

    if "l1" not in _cache:
        _cache["l1"] = _build_launch1()
    nc1 = _cache["l1"]

    xt, w8 = _prep_inputs(test_feature, train_features)
    in_maps = [{"xt": xt, "w": np.ascontiguousarray(w8[c])}
               for c in range(NCORES)]
    res1 = bass_utils.run_bass_kernel_spmd(
        nc1, in_maps, core_ids=list(range(NCORES)), **kw)
    vals = np.stack([r["vals"] for r in res1.results])
    idxs = np.stack([r["idxs"] for r in res1.results])

    dist, gidx = _merge_topk(vals, idxs)
    hw_ns = res1.exec_time_ns or 0

    if not USE_DEVICE_GATHER:
        g = train_labels[np.arange(P)[None, :, None], :, gidx]  # [B,P,K,S]
        grid = _grid_from_retrieved(g.astype(np.int32))
        LAST_HW_NS = hw_ns or None
        return dist, grid

    if "l2" not in _cache:
        _cache["l2"] = _build_launch2()
    nc2 = _cache["l2"]

    # labels transposed to [p, n, s], p-sharded 25/core (padded to 200)
    lab_t = np.zeros((PPAD, N, PS * PS), dtype=np.int32)
    lab_t[:P] = train_labels.transpose(0, 2, 1)
    lab_t = lab_t.reshape(NCORES, PL * N, PS * PS)
    # flat gather row for (p_local, b, k): p_local*N + gidx[b, p, k]
    fi = np.zeros((PPAD, GROWS), dtype=np.int64)
    fi[:P] = gidx.transpose(1, 0, 2).reshape(P, GROWS)
    fi = fi.reshape(NCORES, PL, GROWS)
    fi += (np.arange(PL, dtype=np.int64) * N)[None, :, None]
    fi = fi.reshape(NCORES, NGR)
    off = np.zeros((NCORES, GTILES * 128), dtype=np.int32)
    off[:, :NGR] = fi
    off = np.ascontiguousarray(
        off.reshape(NCORES, GTILES, 128).transpose(0, 2, 1))

    in_maps2 = [{"lab": np.ascontiguousarray(lab_t[c]), "off": off[c]}
                for c in range(NCORES)]
    res2 = bass_utils.run_bass_kernel_spmd(
        nc2, in_maps2, core_ids=list(range(NCORES)), **kw)
    gout = np.stack([r["gout"] for r in res2.results])  # [8, GTILES, 128, S]
    hw_ns += res2.exec_time_ns or 0
    gout = gout.reshape(NCORES, GTILES * 128, PS * PS)[:, :NGR]
    gout = gout.reshape(PPAD, B, K, PS * PS)[:P]
    g = gout.transpose(1, 0, 2, 3)
    grid = _grid_from_retrieved(np.ascontiguousarray(g))
    LAST_HW_NS = hw_ns or None
    return dist, grid
